# revision 1
# baseline (speedup 1.0000x reference)
"""Trainium2 Bass kernel for the Hoyer-spike attention module (B=8,N=1024,C=768,H=12).

Math (per batch, per head): xf = spike1(x); [q|k|v] = xf @ qkv_w.T; ks,vs =
spike2(k),spike2(v) (binary); y = q @ (ks.T @ vs) (exact reassociation of
(q@ks.T)@vs -- no softmax); z = spike3(y) with torch's reshape(B,H,D,N)
reinterpretation; out = z @ proj_w.T + proj_b.

Distribution: data-parallel over B=8 -> one batch per NeuronCore, weights
replicated, no collectives.

Numerics: qkv weights split hi+lo bf16 and accumulated in one PSUM group
(xf binary => products exact; ~17-bit effective weights). M = ks.T@vs is
exact small integers. y-matmul in plain fp32. proj weights single bf16.
BN+Hoyer affine transforms are folded host-side: into the k/v weight rows
(scale) + per-column thresholds, into the x/q copies (per-partition scale)
+ per-partition thresholds -- every spike is a single DVE op.

Layouts: x and weights host-transposed; xfT then serves both as stationary
operand (natural-layout k|v) and moving operand (transposed qT). The torch
reshape shuffle is absorbed into the qT PSUM->SBUF copy via a rearranged
output access pattern, so the y-matmul uses contiguous operands and its
spike lands directly in zT layout for the proj matmul. Odd heads are
re-based to partition 0 with small SBUF->SBUF DMAs (DMA moves across
partitions; DVE cannot).
"""
import sys
sys.path.insert(0, '/opt/trn_rl_repo')
import numpy as np
import ml_dtypes

import concourse.bass as bass
import concourse.mybir as mybir
import concourse.tile as tile
from concourse import bacc

F32 = mybir.dt.float32
BF16 = mybir.dt.bfloat16
FP16 = mybir.dt.float16
AOT = mybir.AluOpType

B, N, C, H, D = 8, 1024, 768, 12, 64
EPS, XS = 1e-5, 1.0
NCORES = 8
BF = np.dtype(ml_dtypes.bfloat16)


def build_nc(rounds=1, upto=5, mul_qt=1, mul_kv=1, mul_z=1, mul_proj=1, mul_dve=1, mul_dma=1):
    nc = bacc.Bacc(None, target_bir_lowering=False)
    xt_d = nc.declare_dram_parameter("xt", [C, N], F32, isOutput=False)
    whi_d = nc.declare_dram_parameter("w_hi", [C, 3 * C], BF16, isOutput=False)
    wlo_d = nc.declare_dram_parameter("w_lo", [C, 3 * C], BF16, isOutput=False)
    phi_d = nc.declare_dram_parameter("p_hi", [C, C], BF16, isOutput=False)
    txa_d = nc.declare_dram_parameter("txA", [128, 6], F32, isOutput=False)
    txt_d = nc.declare_dram_parameter("txT", [128, 6], F32, isOutput=False)
    tkv_d = nc.declare_dram_parameter("tkv", [128, 2 * C], F32, isOutput=False)
    qsc_d = nc.declare_dram_parameter("qsc", [128, 6], F32, isOutput=False)
    tyt_d = nc.declare_dram_parameter("tyT", [128, 6], F32, isOutput=False)
    pb_d = nc.declare_dram_parameter("pb", [128, C], F32, isOutput=False)
    out_d = nc.declare_dram_parameter("out", [N, C], F32, isOutput=True)

    with tile.TileContext(nc) as tc:
        with (
            tc.tile_pool(name="const", bufs=1) as const,
            tc.tile_pool(name="work", bufs=3) as work,
            tc.tile_pool(name="mm", bufs=4, space="PSUM") as mm,
            tc.tile_pool(name="mps", bufs=1, space="PSUM") as mps,
        ):
            # ---- constants ----
            txa = const.tile([128, 6], F32, name="txa")
            txt = const.tile([128, 6], F32, name="txt")
            tkv = const.tile([128, 2 * C], F32, name="tkv")
            qsc = const.tile([128, 6], F32, name="qsc")
            tyt = const.tile([128, 6], F32, name="tyt")
            pb = const.tile([128, C], F32, name="pb")
            nc.sync.dma_start(txa[:], txa_d[:])
            nc.sync.dma_start(txt[:], txt_d[:])

            w_hi = [const.tile([128, 3 * C], BF16, name=f"whi{ck}") for ck in range(6)]
            w_lo = [const.tile([128, 3 * C], BF16, name=f"wlo{ck}") for ck in range(6)]
            p_hi = [const.tile([128, C], BF16, name=f"phi{ck}") for ck in range(6)]

            for _r in range(rounds):
                # ---- phase 1: xT -> spike -> xfT (bf16 binary) ----
                # DMA order follows first use: x chunks + q-columns of w_hi first
                # (phase 2 can start), then w_lo q-cols, then k|v columns, proj
                # weights last.
                xf = [const.tile([128, N], BF16, name=f"xf{ck}_{_r}", tag=f"xf{ck}") for ck in range(6)]
                xtss = []
                for ck in range(6):
                    xts = work.tile([128, N], F32, name=f"xts{ck}_{_r}", tag="xt")
                    xtss.append(xts)
                    nc.gpsimd.dma_start(xts[:], xt_d[ck * 128:(ck + 1) * 128, :])
                    nc.sync.dma_start(w_hi[ck][:, 0:C], whi_d[ck * 128:(ck + 1) * 128, 0:C])
                    nc.vector.tensor_scalar(xf[ck][:], xts[:],
                                            txa[:, ck:ck + 1], txt[:, ck:ck + 1],
                                            AOT.mult, AOT.is_ge)
                nc.sync.dma_start(qsc[:], qsc_d[:])
                for ck in range(6):
                    nc.sync.dma_start(w_lo[ck][:, 0:C], wlo_d[ck * 128:(ck + 1) * 128, 0:C])
                for ck in range(6):
                    nc.sync.dma_start(w_hi[ck][:, C:3 * C],
                                      whi_d[ck * 128:(ck + 1) * 128, C:3 * C])
                for ck in range(6):
                    nc.sync.dma_start(w_lo[ck][:, C:3 * C],
                                      wlo_d[ck * 128:(ck + 1) * 128, C:3 * C])
                nc.sync.dma_start(tkv[:], tkv_d[:])
                nc.sync.dma_start(tyt[:], tyt_d[:])
                for ck in range(6):
                    nc.sync.dma_start(p_hi[ck][:], phi_d[ck * 128:(ck + 1) * 128, :])
                nc.sync.dma_start(pb[:], pb_d[:])

                if upto < 2:
                    nc.sync.dma_start(out_d[0:128, 0:C], xtss[0][:, 0:C])
                    continue
                # ---- phase 2: qT (shuffled layout) = A_o * (Wq @ xfT) ----
                # chunk hp holds heads (2hp, 2hp+1) on partitions 0:64 / 64:128.
                # Shuffled free axis: col m = (n%16)*64 + n//16 so the y-matmul
                # lhsT slices are contiguous.
                qT = [const.tile([128, N], F32, name=f"qT{hp}_{_r}", tag=f"qT{hp}") for hp in range(6)]
                for hp in range(6):
                    for nf in range(2):
                        p = mm.tile([128, 512], F32, name=f"qp{hp}_{nf}_{_r}", tag="mm")
                        for _m in range(mul_qt):
                            for hl, wgt in enumerate((w_hi, w_lo)):
                                for ck in range(6):
                                    nc.tensor.matmul(p[:], wgt[ck][:, hp * 128:(hp + 1) * 128],
                                                     xf[ck][:, nf * 512:(nf + 1) * 512],
                                                     start=(hl == 0 and ck == 0),
                                                     stop=(hl == 1 and ck == 5))
                        src = p[:, :].rearrange("p (a b) -> p a b", a=32)
                        dst = qT[hp][:, :].rearrange("p (b a) -> p a b", b=16)[:, nf * 32:(nf + 1) * 32, :]
                        for _m in range(mul_dve):
                            nc.vector.tensor_scalar(dst, src, qsc[:, hp:hp + 1], None, AOT.mult)

                if upto < 3:
                    nc.sync.dma_start(out_d[0:128, 0:C], qT[0][:, 0:C])
                    continue
                # ---- phase 3: k|v chunks + spikes + M accumulation ----
                m_ps = mps.tile([64, H * D], F32, name=f"m_ps{_r}", tag="m_ps")   # all heads along free
                for nk in range(8):
                    kvs = work.tile([128, 2 * C], BF16, name=f"kvs{nk}_{_r}", tag="kvs")
                    for kvf in range(3):
                        p = mm.tile([128, 512], F32, name=f"kvp{nk}_{kvf}_{_r}", tag="mm")
                        for _m in range(mul_kv):
                            for hl, wgt in enumerate((w_hi, w_lo)):
                                for ck in range(6):
                                    nc.tensor.matmul(p[:], xf[ck][:, nk * 128:(nk + 1) * 128],
                                                     wgt[ck][:, C + kvf * 512: C + (kvf + 1) * 512],
                                                     start=(hl == 0 and ck == 0),
                                                     stop=(hl == 1 and ck == 5))
                        for _m in range(mul_dve):
                            nc.vector.tensor_tensor(kvs[:, kvf * 512:(kvf + 1) * 512], p[:],
                                                    tkv[:, kvf * 512:(kvf + 1) * 512], AOT.is_ge)
                    # PSUM accumulation groups: m_ps spans 2 banks (heads 0-7 /
                    # 8-11); open each bank's group on its first matmul, close on
                    # its last.
                    for h in range(H):
                        nc.tensor.matmul(m_ps[:, h * 64:(h + 1) * 64],
                                         kvs[:, h * 64:(h + 1) * 64],
                                         kvs[:, C + h * 64: C + (h + 1) * 64],
                                         start=(nk == 0 and h in (0, 8)),
                                         stop=(nk == 7 and h in (7, 11)))

                if upto < 4:
                    mdump = work.tile([64, H * D], F32, name=f"mdump{_r}", tag="mdump")
                    nc.vector.tensor_copy(mdump[:], m_ps[:])
                    nc.sync.dma_start(out_d[0:64, 0:H * D], mdump[:, :])
                    continue
                # ---- phase 4: y-matmul -> spike -> zT (head pairs packed via
                # tile_position row+col; HW-validated construct) ----
                m_lo128 = const.tile([128, H * D], F32, name=f"m_lo128_{_r}", tag="m_lo128")
                # copy+partition-shift in column halves: zT for head pairs 0-2
                # can start while pairs 3-5's M is still in flight
                for mh in range(2):
                    cs = slice(mh * 384, (mh + 1) * 384)
                    nc.vector.tensor_copy(m_lo128[0:64, cs], m_ps[:, cs])
                    nc.sync.dma_start(m_lo128[64:128, cs], m_lo128[0:64, cs])
                z_bf = [const.tile([128, N], BF16, name=f"z{hp}_{_r}", tag=f"z{hp}") for hp in range(6)]
                for hp in range(6):
                    hA, hB = 2 * hp, 2 * hp + 1
                    for half in range(2):
                        zp = mm.tile([128, 512], F32, name=f"zp{hp}_{half}_{_r}", tag="mm")
                        for _m in range(mul_z):
                          for q8 in range(8):
                            qb = half * 8 + q8
                            # each (head, q8) slice is written exactly once
                            # (start=True overwrite semantics are per-element on
                            # HW -- validated by probe; skip the sim's coarse
                            # per-zero-region group check, which ignores the
                            # partition base and would false-positive here)
                            nc.tensor.matmul(zp[0:64, q8 * 64:(q8 + 1) * 64],
                                             qT[hp][0:64, qb * 64:(qb + 1) * 64],
                                             m_lo128[0:64, hA * 64:(hA + 1) * 64],
                                             start=True, stop=True,
                                             tile_position=(0, 0),
                                             skip_group_check=True)
                            nc.tensor.matmul(zp[64:128, q8 * 64:(q8 + 1) * 64],
                                             qT[hp][64:128, qb * 64:(qb + 1) * 64],
                                             m_lo128[64:128, hB * 64:(hB + 1) * 64],
                                             start=True, stop=True,
                                             tile_position=(64, 64),
                                             skip_group_check=True)
                        for _m in range(mul_dve):
                            nc.vector.tensor_scalar(
                                z_bf[hp][:, half * 512:(half + 1) * 512], zp[:],
                                tyt[:, hp:hp + 1], None, AOT.is_ge)

                if upto < 5:
                    nc.sync.dma_start(out_d[0:128, 0:C], qT[0][:, 0:C])
                    continue
                # ---- phase 5: out = z @ proj_w.T + pb ----
                for nk in range(8):
                    outs = work.tile([128, C], F32, name=f"outs{nk}_{_r}", tag="outs")
                    for half in range(2):
                        pp = mm.tile([128, 384], F32, name=f"pp{nk}_{half}_{_r}", tag="mm")
                        for _m in range(mul_proj):
                            for hp in range(6):
                                nc.tensor.matmul(pp[:], z_bf[hp][:, nk * 128:(nk + 1) * 128],
                                                 p_hi[hp][:, half * 384:(half + 1) * 384],
                                                 start=(hp == 0), stop=(hp == 5))
                        nc.vector.tensor_tensor(outs[:, half * 384:(half + 1) * 384],
                                                pp[:], pb[:, half * 384:(half + 1) * 384],
                                                AOT.add)
                    nc.sync.dma_start(out_d[nk * 128:(nk + 1) * 128, :], outs[:])

    return nc


def prep_params(inputs):
    """Host-side folding of BN/Hoyer params + weight transposes/splits."""
    d = {k: np.asarray(v, np.float32) for k, v in inputs.items()}

    def fold(p, a):
        s = d[p + '_g'] / np.sqrt(d[p + '_v'] + EPS)
        thr = float(d[a + '_thr'])
        A = s / thr
        Bc = (d[p + '_b'] - d[p + '_m'] * s) / thr
        T2 = XS * d[a + '_run'] - Bc
        return A.astype(np.float32), T2.astype(np.float32)

    A_x, T2_x = fold('n', 'a')
    A_k, T2_k = fold('nk', 'ak')
    A_v, T2_v = fold('nv', 'av')
    A_o, T2_o = fold('no', 'ao')

    Wt = d['qkv_w'].T.copy()                       # [C, 3C]
    colscale = np.concatenate([np.ones(C, np.float32),
                               np.repeat(A_k, D), np.repeat(A_v, D)])
    Wt *= colscale[None, :]
    w_hi = Wt.astype(BF)
    w_lo = (Wt - w_hi.astype(np.float32)).astype(BF)

    p_hi = np.ascontiguousarray(d['proj_w'].T).astype(BF)

    def part6(vec):  # [768] -> [128, 6]; col ck = partition chunk ck
        return np.ascontiguousarray(vec.reshape(6, 128).T)

    return dict(
        w_hi=w_hi, w_lo=w_lo, p_hi=p_hi,
        txA=part6(np.repeat(A_x, D)), txT=part6(np.repeat(T2_x, D)),
        tkv=np.ascontiguousarray(np.broadcast_to(
            np.concatenate([np.repeat(T2_k, D), np.repeat(T2_v, D)]),
            (128, 2 * C))).astype(np.float32),
        qsc=part6(np.repeat(A_o, D)),
        tyT=part6(np.repeat(T2_o, D)),
        pb=np.ascontiguousarray(np.broadcast_to(d['proj_b'], (128, C))).astype(np.float32),
    )


def make_in_maps(inputs):
    shared = prep_params(inputs)
    x = np.asarray(inputs['x'], np.float32)
    return [dict(shared, xt=np.ascontiguousarray(x[c].T)) for c in range(NCORES)]


_CACHE = {}


def _make_executor(nc, n_cores=NCORES):
    """Jitted SPMD executor for the Bass graph (mirrors
    concourse.bass2jax.run_bass_via_pjrt, kept reusable for repeat runs)."""
    import jax
    from jax.sharding import Mesh, PartitionSpec
    from jax.experimental.shard_map import shard_map
    from concourse.bass2jax import (_bass_exec_p, install_neuronx_cc_hook,
                                    partition_id_tensor)
    install_neuronx_cc_hook()
    partition_name = nc.partition_id_tensor.name if nc.partition_id_tensor else None
    in_names, out_names, out_avals, zero_outs = [], [], [], []
    for alloc in nc.m.functions[0].allocations:
        if not isinstance(alloc, mybir.MemoryLocationSet):
            continue
        name = alloc.memorylocations[0].name
        if alloc.kind == "ExternalInput":
            if name != partition_name:
                in_names.append(name)
        elif alloc.kind == "ExternalOutput":
            out_names.append(name)
            shape = tuple(alloc.tensor_shape)
            dtype = mybir.dt.np(alloc.dtype)
            out_avals.append(jax.core.ShapedArray(shape, dtype))
            zero_outs.append(np.zeros(shape, dtype))
    n_params = len(in_names)
    n_outs = len(out_avals)
    all_in_names = list(in_names) + list(out_names)
    if partition_name is not None:
        all_in_names.append(partition_name)

    def _body(*args):
        operands = list(args)
        if partition_name is not None:
            operands.append(partition_id_tensor())
        outs = _bass_exec_p.bind(
            *operands,
            out_avals=tuple(out_avals), in_names=tuple(all_in_names),
            out_names=tuple(out_names), lowering_input_output_aliases=(),
            sim_require_finite=True, sim_require_nnan=True, nc=nc,
        )
        return tuple(outs)

    try:
        devices = jax.devices("axon")[:n_cores]
    except RuntimeError:
        devices = jax.devices()[:n_cores]
    mesh = Mesh(np.asarray(devices), ("core",))
    in_specs = (PartitionSpec("core"),) * (n_params + n_outs)
    out_specs = (PartitionSpec("core"),) * n_outs
    donate = tuple(range(n_params, n_params + n_outs))
    sharded = jax.jit(
        shard_map(_body, mesh=mesh, in_specs=in_specs, out_specs=out_specs,
                  check_rep=False),
        donate_argnums=donate, keep_unused=True,
    )

    def run(in_maps):
        per_core = [[np.asarray(m[n]) for n in in_names] for m in in_maps]
        concat_in = [np.concatenate([per_core[c][i] for c in range(n_cores)], axis=0)
                     for i in range(n_params)]
        concat_zeros = [np.zeros((n_cores * z.shape[0], *z.shape[1:]), z.dtype)
                        for z in zero_outs]
        out_arrs = sharded(*concat_in, *concat_zeros)
        return [
            {name: np.asarray(out_arrs[i]).reshape(n_cores, *out_avals[i].shape)[c]
             for i, name in enumerate(out_names)}
            for c in range(n_cores)
        ], out_arrs

    def run_device_args(concat_in, concat_zeros):
        return sharded(*concat_in, *concat_zeros)

    return run, run_device_args, (in_names, out_names, out_avals, zero_outs, n_params)


def kernel(**inputs) -> np.ndarray:
    if 'exec' not in _CACHE:
        nc = build_nc()
        nc.compile()
        run, run_dev, meta = _make_executor(nc, NCORES)
        _CACHE['exec'] = (nc, run, run_dev, meta)
    nc, run, run_dev, meta = _CACHE['exec']
    in_maps = make_in_maps(inputs)
    results, _ = run(in_maps)
    return np.stack([results[c]['out'] for c in range(NCORES)]).astype(np.float32)

